# revision 1
# baseline (speedup 1.0000x reference)
"""Trainium2 Bass kernel for ComplementaryChannelInteraction.

Pipeline (per sample):
  1x1 conv (+folded BN1) -> ReLU -> channel attention softmax(-x@xT) ->
  3x3 conv (+folded BN2) -> ReLU -> global avg pool -> FC -> ReLU -> FC

Sharding: pure data parallel, B=128 -> 16 samples on each of 8 cores,
all params replicated.

Precision: conv1x1 + x@xT in bf16 (f32 PSUM), softmax numerator E and
the attention-apply in fp8 e4m3 (E is scaled by 128 via a ln(128) fold
into the exp bias so its values sit in fp8's normal range), 3x3 conv
in fp8 DoubleRow (2x PE throughput).  fp8 weight-quantization error is
neutralized by a mean-shift split y = c + y': c is a per-channel
constant from sample 0 of each core; conv3(c) is computed once at
higher effective precision (fp8 weights + fp8 weight-residual second
pass) and streamed into each sample's PSUM via one identity matmul, so
the weight error only couples to the small zero-centered residual y'.
Measured end-to-end ~6e-3 rel err (gate 2e-2).

conv3 layout: y lives in a flat 16x16 zero-padded image (row stride
16), two samples packed per 512-wide row, so every 3x3 tap is one
contiguous 478-column stream and both samples of a pair share a single
DoubleRow weight load.  Wrapped/garbage PSUM columns are never read.

Schedule: pair-phase-major (conv1 pair -> fT both -> xxt both -> ET
both -> y both), with conv3 deferred one pair so the PE never waits on
the vector-engine ypad quantization writes.
"""
import contextlib
import ctypes
import math
import sys
import types

import numpy as np
import ml_dtypes

import concourse.bass as bass
import concourse.tile as tile
import concourse.mybir as mybir
from concourse import bacc
from concourse.bass_utils import run_bass_kernel_spmd

dt = mybir.dt
F32, BF16, FP8 = dt.float32, dt.bfloat16, dt.float8e4
AF = mybir.ActivationFunctionType
ALU = mybir.AluOpType
AX = mybir.AxisListType
DR = mybir.MatmulPerfMode.DoubleRow

N_CORES = 8
B, CIN, C, H, W, NCOUT = 128, 2048, 512, 14, 14, 200
P = H * W            # 196
BPC = B // N_CORES   # 16 samples per core
KC = CIN // 128      # 16 contraction chunks for conv1
CC = C // 128        # 4 channel chunks
PCH = [(0, 128), (128, 68)]  # spatial chunks of 196: (offset, size)
EPS = 1e-5
SW = 64.0            # fp8 scale for w3 (and its residual)
SY = 32.0            # fp8 scale for y'
SE = 128.0           # fp8 scale for E (folded into exp bias as ln SE)
LNSE = math.log(SE)
DESCALE = 1.0 / (SW * SY)
STREAM = 478         # conv3 stream: covers both samples' 222-wide windows

# ---------------------------------------------------------------- compat shims


def _install_drain_patch():
    """walrus here allows only ONE sync-wait per Drain; split the Tile
    kernel-tail drain into a chain of single-wait drains."""

    def _split_drain_and_barrier(self, tick_clock, wait_clock):
        from concourse.tile import ScopedClock

        drain_inst = self.nc.sync.drain()
        wait_clock.add_sem_waits(
            drain_inst.ins, ScopedClock({None: tick_clock.global_clock})
        )
        si = drain_inst.ins.sync_info
        waits = list(si.on_wait) if si is not None else []
        if len(waits) > 1:
            drain_inst.ins.sync_info = mybir.SyncInfo(
                on_wait=waits[:1], on_update=list(si.on_update)
            )
            for i in range(1, len(waits)):
                extra = self.nc.sync.drain()
                extra.ins.sync_info = mybir.SyncInfo(
                    on_wait=waits[i : i + 1], on_update=[]
                )
        self.nc.all_engine_barrier()
        assert self.sems is not None
        popped = self.nc._tile_sem_poison_stack.pop()
        assert popped is self._sem_poison
        self.nc.clear_and_free_semaphores(list(self.sems.allocated().values()))
        self.nc.all_engine_barrier()

    tile.TileContext._drain_and_barrier = _split_drain_and_barrier


def _install_ntff_hook(so_path="/opt/axon/libaxon_pjrt.so"):
    """antenv.axon_hooks is missing in this image; recreate it so
    trace=True (NTFF profiling) works instead of crashing on import."""
    if "antenv.axon_hooks" in sys.modules:
        return
    mod = types.ModuleType("antenv.axon_hooks")
    state = {"hook": None}
    mod.set_axon_ntff_profile_hook = lambda h: state.__setitem__("hook", h)
    mod.get_axon_ntff_profile_hook = lambda: state["hook"]
    sys.modules["antenv.axon_hooks"] = mod
    try:
        import antenv

        antenv.axon_hooks = mod
    except ImportError:
        pass
    try:
        lib = ctypes.CDLL(so_path)
        if not hasattr(lib, "axon_start_nrt_profile"):
            return
        lib.axon_start_nrt_profile.argtypes = [
            ctypes.POINTER(ctypes.c_int64),
            ctypes.c_size_t,
        ]
        lib.axon_start_nrt_profile.restype = ctypes.c_int64
        lib.axon_stop_nrt_profile.argtypes = [ctypes.c_char_p]
        lib.axon_stop_nrt_profile.restype = ctypes.c_int64
    except OSError:
        return

    @contextlib.contextmanager
    def _hook(output_dir, device_ids):
        import jax

        jax.devices()
        if device_ids:
            ids = (ctypes.c_int64 * len(device_ids))(*device_ids)
            rc = lib.axon_start_nrt_profile(ids, len(device_ids))
        else:
            rc = lib.axon_start_nrt_profile(None, 0)
        if rc != 0:
            raise RuntimeError(f"axon_start_nrt_profile rc={rc}")
        try:
            yield
        finally:
            n = lib.axon_stop_nrt_profile(str(output_dir).encode())
            if n < 0:
                raise RuntimeError(f"axon_stop_nrt_profile rc={n}")
            print(f"profile: {n} file(s) written to {output_dir}", file=sys.stderr)

    state["hook"] = _hook


def install_shims():
    _install_drain_patch()
    _install_ntff_hook()


# ---------------------------------------------------------------- bass program


def build_program(n_samples=BPC):
    install_shims()
    nc = bacc.Bacc(
        "TRN2", target_bir_lowering=False, debug=False, num_devices=N_CORES
    )

    feat_d = nc.dram_tensor("feat", [n_samples, CIN, P], BF16, kind="ExternalInput")
    wpT_d = nc.dram_tensor("wpT", [CIN, C], BF16, kind="ExternalInput")
    t1c_d = nc.dram_tensor("t1c", [128, CC], F32, kind="ExternalInput")
    w3q_d = nc.dram_tensor("w3q", [128, 2, 2, 9, C], FP8, kind="ExternalInput")
    dw_d = nc.dram_tensor("dw", [128, 2, 2, 9, C], FP8, kind="ExternalInput")
    t2_d = nc.dram_tensor("t2", [CC, 128], F32, kind="ExternalInput")
    fc1_d = nc.dram_tensor("fc1", [CC, 128, NCOUT], F32, kind="ExternalInput")
    fc1b_d = nc.dram_tensor("fc1b", [2, 100], F32, kind="ExternalInput")
    fc2_d = nc.dram_tensor("fc2", [2, 100, NCOUT], F32, kind="ExternalInput")
    fc2b_d = nc.dram_tensor("fc2b", [2, 100], F32, kind="ExternalInput")
    identb_d = nc.dram_tensor("identb", [128, 128], BF16, kind="ExternalInput")
    ident8_d = nc.dram_tensor("ident8", [128, 128], FP8, kind="ExternalInput")
    out_d = nc.dram_tensor("out", [n_samples, NCOUT], F32, kind="ExternalOutput")

    with tile.TileContext(nc) as tc:
        with contextlib.ExitStack() as ctx:
            wpool = ctx.enter_context(tc.tile_pool(name="weights", bufs=1))
            featp = ctx.enter_context(tc.tile_pool(name="featp", bufs=2 * KC))
            xcmp = ctx.enter_context(tc.tile_pool(name="xcmp", bufs=4))
            fTp = ctx.enter_context(tc.tile_pool(name="fTp", bufs=3))
            Ep = ctx.enter_context(tc.tile_pool(name="Ep", bufs=2))
            smallp = ctx.enter_context(tc.tile_pool(name="smallp", bufs=3))
            zscrp = ctx.enter_context(tc.tile_pool(name="zscrp", bufs=2))
            ps_c1 = ctx.enter_context(tc.tile_pool(name="ps_c1", bufs=2, space="PSUM"))
            ps_xxt = ctx.enter_context(tc.tile_pool(name="ps_xxt", bufs=3, space="PSUM"))
            ps_sm = ctx.enter_context(tc.tile_pool(name="ps_sm", bufs=2, space="PSUM"))
            ps_z = ctx.enter_context(tc.tile_pool(name="ps_z", bufs=1, space="PSUM"))

            # ---- startup-critical loads: interleave per-k wpT chunks with
            # pair-0 feature chunks so the first conv1 matmul only waits for
            # a fraction of the parameter set.
            def load_pair(g):
                tiles = []
                for k in range(KC):
                    ft = featp.tile([128, 2, P], BF16, tag="feat")
                    nc.sync.dma_start(
                        ft[:],
                        feat_d[2 * g : 2 * g + 2, k * 128 : (k + 1) * 128, :]
                        .rearrange("s c p -> c s p"),
                    )
                    tiles.append(ft)
                return tiles

            wpT = []
            feat0 = []
            for k in range(KC):
                wt = wpool.tile([128, C], BF16, tag=f"wpT{k}")
                nc.sync.dma_start(wt[:], wpT_d[k * 128 : (k + 1) * 128, :])
                wpT.append(wt)
                ft = featp.tile([128, 2, P], BF16, tag="feat")
                nc.sync.dma_start(
                    ft[:],
                    feat_d[0:2, k * 128 : (k + 1) * 128, :].rearrange("s c p -> c s p"),
                )
                feat0.append(ft)

            identb = wpool.tile([128, 128], BF16, tag="identb")
            nc.sync.dma_start(identb[:], identb_d[:])
            t1c = wpool.tile([128, CC], F32, tag="t1c")
            nc.sync.dma_start(t1c[:], t1c_d[:])

            # persistent fp8 padded-y tiles: [p, jp, jj, v, 16, 16], flat
            # per-sample image = 16 rows x 16 cols, borders stay zero.
            ypads = []
            for par in range(2):
                yp = wpool.tile([128, 2, 2, 2, 16, 16], FP8, tag=f"ypad{par}")
                nc.vector.memset(yp[:], 0.0)
                ypads.append(yp)
            # correction row per i-chunk, duplicated at both sample offsets
            corr_sb = wpool.tile([128, CC, 512], BF16, tag="corr")
            nc.vector.memset(corr_sb[:], 0.0)
            # c-field (constant per channel, interior only), fp8, SY-scaled
            cpad = wpool.tile([128, 2, 2, 256], FP8, tag="cpad")
            nc.vector.memset(cpad[:], 0.0)
            # scratch for sample-0 raw y-psum (pre-quantization) + c vectors
            yscr = wpool.tile([128, CC, P], F32, tag="yscr")
            c32 = wpool.tile([128, CC], F32, tag="c32")
            c8 = wpool.tile([128, CC], FP8, tag="c8")
            csc = wpool.tile([128, CC], F32, tag="csc")

            # accumulated pooled z for the whole per-core batch
            zall = wpool.tile([128, CC, n_samples], F32, tag="zall")

            state = {}

            def emit_conv3_group(gi, i):
                """One output-channel chunk of the 3x3 conv for pair gi:
                fp8 DoubleRow + exact-shift correction, both samples in one
                478-wide stream.  Emitted between attention phases of the
                NEXT pair so the PE never idles while attention drains."""
                ypad = ypads[gi % 2]
                w3q = state["w3q"]
                pz = ps_z.tile([128, 512], F32, tag="z")
                nc.tensor.matmul(
                    pz[:, 0:STREAM],
                    identb[:],
                    corr_sb[:, i, 0:STREAM],
                    start=True,
                    stop=False,
                )
                for tap in range(9):
                    toff = (tap // 3) * 16 + (tap % 3)
                    for jp in range(2):
                        nc.tensor.matmul(
                            pz[:, 0:STREAM],
                            w3q[:, jp, :, tap, i * 128 : (i + 1) * 128],
                            ypad[:, jp, :, :, :, :]
                            .rearrange("p jj v h x -> p jj (v h x)")
                            [:, :, toff : toff + STREAM],
                            start=False,
                            stop=(tap == 8 and jp == 1),
                            perf_mode=DR,
                        )
                for v in range(2):
                    s = 2 * gi + v
                    zscr = zscrp.tile([128, H, W], BF16, tag="zscr")
                    nc.scalar.activation(
                        zscr[:],
                        pz[:].rearrange("p (v h x) -> p v h x", v=2, h=16)
                        [:, v, 0:14, 0:14],
                        AF.Relu,
                        bias=state["t2sb"][:, i : i + 1],
                        scale=DESCALE,
                        accum_out=zall[:, i, s : s + 1],
                    )

            n_pairs = n_samples // 2
            for gi in range(n_pairs):
                ypad = ypads[gi % 2]
                feat = feat0 if gi == 0 else load_pair(gi)
                # conv3 groups of the previous pair, interleaved as PE filler
                filler = (
                    [lambda i=i: emit_conv3_group(gi - 1, i) for i in range(CC)]
                    if gi > 0 else [lambda: None] * CC
                )

                # ---- 1x1 conv, channel-major, both samples in one stream
                xcm = []
                for v in range(2):
                    xcm_t = xcmp.tile([128, CC, P], BF16, tag="xcm")
                    xcm.append(xcm_t)
                for i in range(CC):
                    pc = ps_c1.tile([128, 2, P], F32, tag="c1")
                    for k in range(KC):
                        nc.tensor.matmul(
                            pc[:],
                            wpT[k][:, i * 128 : (i + 1) * 128],
                            feat[k][:],
                            start=(k == 0),
                            stop=(k == KC - 1),
                        )
                    for v in range(2):
                        nc.scalar.activation(
                            xcm[v][:, i, :], pc[:, v], AF.Relu,
                            bias=t1c[:, i : i + 1],
                        )

                if gi == 0:
                    # deferred param loads: issued after pair-0 conv1 so they
                    # don't compete with the startup-critical DMAs above.
                    w3q = wpool.tile([128, 2, 2, 9, C], FP8, tag="w3q")
                    nc.sync.dma_start(w3q[:], w3q_d[:])
                    state["w3q"] = w3q
                    dwq = wpool.tile([128, 2, 2, 9, C], FP8, tag="dwq")
                    nc.sync.dma_start(dwq[:], dw_d[:])
                    state["dwq"] = dwq
                    t2sb = wpool.tile([128, CC], F32, tag="t2sb")
                    nc.sync.dma_start(t2sb[:], t2_d[:].rearrange("j p -> p j"))
                    state["t2sb"] = t2sb
                    fc1sb = wpool.tile([128, CC, NCOUT], F32, tag="fc1sb")
                    nc.sync.dma_start(fc1sb[:], fc1_d[:].rearrange("j p o -> p j o"))
                    state["fc1sb"] = fc1sb
                    fc1bsb = wpool.tile([128, 2], F32, tag="fc1bsb")
                    nc.sync.dma_start(fc1bsb[:100, :], fc1b_d[:].rearrange("m p -> p m"))
                    state["fc1bsb"] = fc1bsb
                    fc2sb = wpool.tile([128, 2, NCOUT], F32, tag="fc2sb")
                    nc.sync.dma_start(fc2sb[:100, :, :], fc2_d[:].rearrange("m p o -> p m o"))
                    state["fc2sb"] = fc2sb
                    fc2bsb = wpool.tile([128, 2], F32, tag="fc2bsb")
                    nc.sync.dma_start(fc2bsb[:100, :], fc2b_d[:].rearrange("m p -> p m"))
                    state["fc2bsb"] = fc2bsb

                # ---- transpose to spatial-major fT [196, 2, 512], both
                # samples; psum drains alternate scalar/vector so neither
                # engine falls behind the PE.
                fTs = []
                for v in range(2):
                    fT_sb = fTp.tile([128, 2, C], BF16, tag="fT")
                    fTs.append(fT_sb)
                    for i in range(CC):
                        for m, (po, pn) in enumerate(PCH):
                            ptF = ps_sm.tile([128, 128], BF16, tag="small")
                            nc.tensor.transpose(
                                ptF[:pn, :],
                                xcm[v][:, i, po : po + pn],
                                identb[:],
                            )
                            eng = nc.vector if (i + m) % 2 else nc.scalar
                            if eng is nc.scalar:
                                nc.scalar.copy(
                                    fT_sb[:pn, m, i * 128 : (i + 1) * 128],
                                    ptF[:pn, :],
                                )
                            else:
                                nc.vector.tensor_copy(
                                    fT_sb[:pn, m, i * 128 : (i + 1) * 128],
                                    ptF[:pn, :],
                                )

                filler[0]()

                # ---- xxt + softmax numerator E per sample, with conv3
                # filler between the two samples.  Row-shifted:
                # E[c,d] = exp(-xxt[c,d]+m_c) (required: some rows have
                # min xxt > 87; exp(-xxt) would underflow to a zero row).
                Es = []
                zi32s = []
                zis = []
                for v in range(2):
                    fT_sb = fTs[v]
                    E_sb = Ep.tile([128, CC, C], BF16, tag="E")
                    Es.append(E_sb)
                    zrow = smallp.tile([128, CC], F32, tag="zrow")
                    zinv = smallp.tile([128, CC], F32, tag="zinv")
                    zis.append(zinv)
                    zinv32 = smallp.tile([128, CC], F32, tag="zinv32")
                    zi32s.append(zinv32)
                    mrow = smallp.tile([128, CC], F32, tag="mrow")
                    for i in range(CC):
                        pxxt = ps_xxt.tile([128, C], F32, tag="xxt")
                        for m, (po, pn) in enumerate(PCH):
                            nc.tensor.matmul(
                                pxxt[:],
                                fT_sb[:pn, m, i * 128 : (i + 1) * 128],
                                fT_sb[:pn, m, :],
                                start=(m == 0),
                                stop=(m == 1),
                            )
                        nc.vector.tensor_reduce(
                            out=mrow[:, i : i + 1], in_=pxxt[:], op=ALU.min, axis=AX.X
                        )
                        nc.scalar.activation(
                            E_sb[:, i, :],
                            pxxt[:],
                            AF.Exp,
                            bias=mrow[:, i : i + 1],
                            scale=-1.0,
                            accum_out=zrow[:, i : i + 1],
                        )
                    nc.vector.reciprocal(zinv[:], zrow[:])
                    nc.vector.tensor_scalar_mul(zinv32[:], zinv[:], SY)
                    filler[1 + v]()

                # ---- ET via PE transposes (E is not symmetric: per-row
                # shift), then y = (E @ x)/Z, per sample; conv3 filler
                # between the samples.  Copies alternate scalar/vector.
                for v in range(2):
                    s = 2 * gi + v
                    E_sb = Es[v]
                    zinv32 = zi32s[v]
                    ET_sb = Ep.tile([128, CC, C], BF16, tag="ET")
                    for i in range(CC):
                        for j in range(CC):
                            ptE = ps_sm.tile([128, 128], BF16, tag="small")
                            nc.tensor.transpose(
                                ptE[:],
                                E_sb[:, j, i * 128 : (i + 1) * 128],
                                identb[:],
                            )
                            if (i + j) % 2:
                                nc.scalar.copy(
                                    ET_sb[:, i, j * 128 : (j + 1) * 128], ptE[:]
                                )
                            else:
                                nc.vector.tensor_copy(
                                    ET_sb[:, i, j * 128 : (j + 1) * 128], ptE[:]
                                )
                    for i in range(CC):
                        py = ps_sm.tile([128, P], F32, tag="small")
                        for j in range(CC):
                            nc.tensor.matmul(
                                py[:],
                                ET_sb[:, j, i * 128 : (i + 1) * 128],
                                xcm[v][:, j, :],
                                start=(j == 0),
                                stop=(j == CC - 1),
                            )
                        if s == 0:
                            # raw y-psum parked in SBUF; the shift constant c
                            # isn't known yet.
                            nc.scalar.copy(yscr[:, i, :], py[:])
                        else:
                            nc.vector.tensor_scalar(
                                out=ypad[:, i // 2, i % 2, v, 1:15, 1:15],
                                in0=py[:].rearrange("p (h w) -> p h w", h=H),
                                scalar1=zinv32[:, i : i + 1],
                                scalar2=c32[:, i : i + 1],
                                op0=ALU.mult,
                                op1=ALU.subtract,
                            )
                    if v == 0:
                        filler[3]()

                    if s == 0:
                        # ---- one-time: c from sample 0, correction conv
                        ysum = smallp.tile([128, CC], F32, tag="ysum")
                        for i in range(CC):
                            nc.vector.tensor_reduce(
                                out=ysum[:, i : i + 1], in_=yscr[:, i, :],
                                op=ALU.add, axis=AX.X,
                            )
                        # csc = SY * mean_n(y) = ysum * zinv * (SY/196)
                        nc.vector.tensor_tensor(
                            out=csc[:], in0=ysum[:], in1=zis[0][:], op=ALU.mult
                        )
                        nc.vector.tensor_scalar_mul(csc[:], csc[:], SY / float(P))
                        nc.vector.tensor_copy(c8[:], csc[:])   # quantize
                        nc.vector.tensor_copy(c32[:], c8[:])   # exact dequant
                        # c-field (SY-scaled, fp8-exact) interiors
                        for i in range(CC):
                            nc.scalar.activation(
                                cpad[:, i // 2, i % 2, :]
                                .rearrange("p (h x) -> p h x", h=16)[:, 1:15, 1:15],
                                yscr[:, i, 0:P].rearrange("p (h w) -> p h w", h=H),
                                AF.Identity,
                                bias=c32[:, i : i + 1],
                                scale=0.0,
                            )
                        # corr = conv3(cfield, w3q + dw), both fp8 DoubleRow;
                        # result lands in the same SW*SY scale as the main conv.
                        for i in range(CC):
                            pcr = ps_sm.tile([128, 224], F32, tag="small")
                            first = True
                            for wt_ in (state["w3q"], state["dwq"]):
                                for tap in range(9):
                                    toff = (tap // 3) * 16 + (tap % 3)
                                    for jp in range(2):
                                        nc.tensor.matmul(
                                            pcr[:, 0:222],
                                            wt_[:, jp, :, tap, i * 128 : (i + 1) * 128],
                                            cpad[:, jp, :, toff : toff + 222],
                                            start=first,
                                            stop=(wt_ is state["dwq"]
                                                  and tap == 8 and jp == 1),
                                            perf_mode=DR,
                                        )
                                        first = False
                            for voff in (0, 256):
                                nc.scalar.copy(
                                    corr_sb[:, i, voff : voff + 222], pcr[:, 0:222]
                                )
                        # now quantize sample-0's y
                        for i in range(CC):
                            nc.vector.tensor_scalar(
                                out=ypad[:, i // 2, i % 2, v, 1:15, 1:15],
                                in0=yscr[:, i, :].rearrange("p (h w) -> p h w", h=H),
                                scalar1=zi32s[0][:, i : i + 1],
                                scalar2=c32[:, i : i + 1],
                                op0=ALU.mult,
                                op1=ALU.subtract,
                            )

            for i in range(CC):
                emit_conv3_group(n_pairs - 1, i)

            # ---- FC head over the whole per-core batch
            h_sb = smallp.tile([128, 2, n_samples], F32, tag="h")
            for m in range(2):
                ph = ps_sm.tile([128, n_samples], F32, tag="small")
                for j in range(CC):
                    nc.tensor.matmul(
                        ph[:100, :],
                        state["fc1sb"][:, j, m * 100 : (m + 1) * 100],
                        zall[:, j, :],
                        start=(j == 0),
                        stop=(j == CC - 1),
                    )
                nc.scalar.activation(
                    h_sb[:100, m, :], ph[:100, :], AF.Relu,
                    bias=state["fc1bsb"][:100, m : m + 1],
                )
            sf_sb = smallp.tile([128, 2, n_samples], F32, tag="sf")
            for m2 in range(2):
                psf = ps_sm.tile([128, n_samples], F32, tag="small")
                for m in range(2):
                    nc.tensor.matmul(
                        psf[:100, :],
                        state["fc2sb"][:100, m, m2 * 100 : (m2 + 1) * 100],
                        h_sb[:100, m, :],
                        start=(m == 0),
                        stop=(m == 1),
                    )
                nc.scalar.activation(
                    sf_sb[:100, m2, :],
                    psf[:100, :],
                    AF.Identity,
                    bias=state["fc2bsb"][:100, m2 : m2 + 1],
                )
                nc.sync.dma_start(
                    out_d[:, m2 * 100 : (m2 + 1) * 100].rearrange("b o -> o b"),
                    sf_sb[:100, m2, :],
                )

    nc.compile()
    return nc


# ---------------------------------------------------------------- host wrapper

_prog_cache = {}


def _get_program(n_samples=BPC):
    if n_samples not in _prog_cache:
        _prog_cache[n_samples] = build_program(n_samples)
    return _prog_cache[n_samples]


def _cast8(x):
    return np.clip(x, -240.0, 240.0).astype(ml_dtypes.float8_e4m3)


def prepare_host_inputs(inputs):
    """Fold BN into weights, build the per-core replicated param arrays."""
    s1 = inputs["bn1_gamma"] / np.sqrt(inputs["bn1_var"] + EPS)
    t1 = (inputs["b_reduce"] - inputs["bn1_mean"]) * s1 + inputs["bn1_beta"]
    Wp = inputs["w_reduce"].reshape(C, CIN) * s1[:, None]
    wpT = np.ascontiguousarray(Wp.T).astype(ml_dtypes.bfloat16)  # [2048, 512]
    t1c = np.ascontiguousarray(t1.reshape(CC, 128).T)            # [128, CC]

    s2 = inputs["bn2_gamma"] / np.sqrt(inputs["bn2_var"] + EPS)
    t2 = (inputs["b3"] - inputs["bn2_mean"]) * s2 + inputs["bn2_beta"]
    w3p = inputs["w3"] * s2[:, None, None, None]            # [co, ci, ky, kx]
    w3s = (w3p * SW).astype(np.float32)
    w3q8 = _cast8(w3s)
    dw8 = _cast8(w3s - w3q8.astype(np.float32))
    # [co, ci, tap] -> [ci, tap, co] -> [jp, jj, p, tap, co] -> [p, jp, jj, tap, co]
    w3q_l = np.ascontiguousarray(
        w3q8.reshape(C, C, 9).transpose(1, 2, 0).reshape(2, 2, 128, 9, C)
        .transpose(2, 0, 1, 3, 4)
    )
    dw_l = np.ascontiguousarray(
        dw8.reshape(C, C, 9).transpose(1, 2, 0).reshape(2, 2, 128, 9, C)
        .transpose(2, 0, 1, 3, 4)
    )
    t2_a = np.ascontiguousarray(t2.reshape(CC, 128))

    fc1p = (inputs["fc1_w"] / float(P)).astype(np.float32)  # fold 1/196 mean
    fc1 = np.ascontiguousarray(fc1p.T.reshape(CC, 128, NCOUT))
    fc1b = np.ascontiguousarray(inputs["fc1_b"].reshape(2, 100))
    fc2 = np.ascontiguousarray(inputs["fc2_w"].T.reshape(2, 100, NCOUT))
    fc2b = np.ascontiguousarray(inputs["fc2_b"].reshape(2, 100))
    identb = np.eye(128, dtype=ml_dtypes.bfloat16)
    ident8 = np.eye(128, dtype=ml_dtypes.float8_e4m3)
    return {
        "identb": identb,
        "ident8": ident8,
        "wpT": wpT,
        "t1c": t1c,
        "w3q": w3q_l,
        "dw": dw_l,
        "t2": t2_a,
        "fc1": fc1,
        "fc1b": fc1b,
        "fc2": fc2,
        "fc2b": fc2b,
    }


def run(inputs, n_samples=BPC, n_cores=N_CORES, trace=False):
    nc = _get_program(n_samples)
    params = prepare_host_inputs(inputs)
    feat = np.asarray(inputs["feature"], np.float32).reshape(B, CIN, P).astype(ml_dtypes.bfloat16)
    in_maps = []
    for c in range(n_cores):
        m = dict(params)
        m["feat"] = np.ascontiguousarray(feat[c * n_samples : (c + 1) * n_samples])
        in_maps.append(m)
    res = run_bass_kernel_spmd(nc, in_maps, list(range(n_cores)), trace=trace)
    out = np.concatenate([res.results[c]["out"] for c in range(n_cores)], axis=0)
    return out, res


def kernel(**inputs):
    inputs = {k: np.asarray(v) for k, v in inputs.items()}
    out, _ = run(inputs)
    return out.astype(np.float32)

